# revision 15
# baseline (speedup 1.0000x reference)
"""AMGSRN (multi-grid encoding + MLP decoder) on 8 Trainium2 NeuronCores.

Strategy (data-parallel over the point batch, grids/weights replicated):
  - Host precomputes a "V8" corner table per grid: for every padded cell c
    (66^3 cells per grid) it stores the 16 values {corner(dz,dy,dx), feature f}
    contiguously (32B in bf16).  One trilinear sample then needs exactly ONE
    32B indirect-DMA gather per (point, grid).
  - Device per chunk of 128 points: PE computes the affine transform straight
    into padded-grid coordinates, ACT clamps, DVE computes cell indices +
    fractional weights, one indirect DMA gathers 128x64 corner packs, DVE does
    the factored trilinear combine (z, then y, then x), PE transposes the
    [point, feature] tile and runs the 128->64->64->1 MLP with fused
    bias+ReLU on ACT.
  - No collectives: each core owns 32768 points end-to-end.
"""

import sys

sys.path.insert(0, "/opt/trn_rl_repo")

import numpy as np

import concourse.bass as bass
import concourse.bacc as bacc_mod
import concourse.mybir as mybir
import concourse.tile as tile
from concourse.masks import make_identity

# problem constants (hardcoded per harness rules)
G = 64          # grids
F = 2           # features per grid
R = 64          # grid resolution
N = 262144      # total points
NCORES = 8
NP = N // NCORES            # points per core
PAD = 66                    # padded cell axis (cell' in [0,65])
CELLS = PAD * PAD * PAD     # 287496 cells per grid
CHUNK = 128
SUB_PER_SUPER = 4
SUPER = CHUNK * SUB_PER_SUPER   # 512 points
NSUPER = NP // SUPER

FP32 = mybir.dt.float32
BF16 = mybir.dt.bfloat16
I32 = mybir.dt.int32


def _register_const(nc, dtype, value):
    t = nc.alloc_sbuf_tensor(f"const-{dtype.name}-{value}", [128, 1], dtype)
    nc.gpsimd.memset(t.ap(), value)
    nc.const_aps.aps[(dtype, value)] = t.ap()


def build_bass(np_points=NP, debug=False):
    nsuper = np_points // SUPER
    nc = bacc_mod.Bacc()
    dbg = {}
    if debug:
        dbg["coor"] = nc.declare_dram_parameter("dbg_coor", [128, SUB_PER_SUPER * 3 * G], FP32, isOutput=True)
        dbg["frac"] = nc.declare_dram_parameter("dbg_frac", [128, SUB_PER_SUPER * 3 * G], FP32, isOutput=True)
        dbg["idx"] = nc.declare_dram_parameter("dbg_idx", [128, SUB_PER_SUPER * G], I32, isOutput=True)
        dbg["gat"] = nc.declare_dram_parameter("dbg_gat", [128, SUB_PER_SUPER * G * 16], BF16, isOutput=True)
        dbg["feats"] = nc.declare_dram_parameter("dbg_feats", [128, 128], BF16, isOutput=True)
        dbg["featsT"] = nc.declare_dram_parameter("dbg_featsT", [128, 128], BF16, isOutput=True)

    xh_ext = nc.declare_dram_parameter("xh", [4, np_points], FP32, isOutput=False)
    m2t_ext = nc.declare_dram_parameter("m2t", [4, 3 * G], FP32, isOutput=False)
    v8_ext = nc.declare_dram_parameter("v8", [G * CELLS, 16], BF16, isOutput=False)
    goff_ext = nc.declare_dram_parameter("goff", [128, G], I32, isOutput=False)
    w0_ext = nc.declare_dram_parameter("w0", [G * F, 64], BF16, isOutput=False)
    w1_ext = nc.declare_dram_parameter("w1", [64, 64], BF16, isOutput=False)
    w2_ext = nc.declare_dram_parameter("w2", [64, 1], BF16, isOutput=False)
    b0_ext = nc.declare_dram_parameter("b0", [64, 1], FP32, isOutput=False)
    b1_ext = nc.declare_dram_parameter("b1", [64, 1], FP32, isOutput=False)
    b2_ext = nc.declare_dram_parameter("b2", [1, 1], FP32, isOutput=False)
    out_ext = nc.declare_dram_parameter("out", [np_points, 1], FP32, isOutput=True)

    Relu = mybir.ActivationFunctionType.Relu
    Copy = mybir.ActivationFunctionType.Copy
    mult = mybir.AluOpType.mult
    add = mybir.AluOpType.add
    subtract = mybir.AluOpType.subtract

    with tile.TileContext(nc) as tc:
        with (
            tc.tile_pool(name="const", bufs=1) as cpool,
            tc.tile_pool(name="sb", bufs=2) as pool,
            tc.tile_pool(name="psum", bufs=2, space="PSUM") as pp,
            tc.tile_pool(name="psum1", bufs=1, space="PSUM") as pp1,
        ):
            # ---- persistent constants ----
            m2t = cpool.tile([4, 3 * G], FP32)
            nc.sync.dma_start(out=m2t[:], in_=m2t_ext[:])
            goff = cpool.tile([128, G], I32)
            nc.sync.dma_start(out=goff[:], in_=goff_ext[:])
            w0 = cpool.tile([G * F, 64], BF16)
            nc.sync.dma_start(out=w0[:], in_=w0_ext[:])
            w1 = cpool.tile([64, 64], BF16)
            nc.sync.dma_start(out=w1[:], in_=w1_ext[:])
            w2 = cpool.tile([64, 1], BF16)
            nc.sync.dma_start(out=w2[:], in_=w2_ext[:])
            b0 = cpool.tile([64, 1], FP32)
            nc.sync.dma_start(out=b0[:], in_=b0_ext[:])
            b1 = cpool.tile([64, 1], FP32)
            nc.sync.dma_start(out=b1[:], in_=b1_ext[:])
            b2 = cpool.tile([1, 1], FP32)
            nc.sync.dma_start(out=b2[:], in_=b2_ext[:])
            ident = cpool.tile([128, 128], BF16)
            make_identity(nc, ident[:])

            for s in range(nsuper):
                # ---- load this super-chunk's points ----
                xh = pool.tile([4, SUPER], FP32, tag="xh")
                nc.sync.dma_start(out=xh[:], in_=xh_ext[:, s * SUPER:(s + 1) * SUPER])

                # per-super working tiles
                coor = pool.tile([128, SUB_PER_SUPER * 3 * G], FP32, tag="coor")
                cflt = pool.tile([128, SUB_PER_SUPER * 3 * G], FP32, tag="cflt")
                frac = pool.tile([128, SUB_PER_SUPER * 3 * G], FP32, tag="frac")
                gfrac = pool.tile([128, SUB_PER_SUPER * 3 * G], FP32, tag="gfrac")
                idx = pool.tile([128, SUB_PER_SUPER * G], I32, tag="idx")
                idxf = pool.tile([128, SUB_PER_SUPER * G], FP32, tag="idxf")

                for c in range(SUB_PER_SUPER):
                    co = c * 3 * G  # column offset in the (sub, ax, g) layout
                    # transform: [4,128].T @ [4,192] -> [128, 192] (ax-major: x|y|z)
                    ps = pp.tile([128, 3 * G], FP32, tag="ps_tr")
                    nc.tensor.matmul(
                        ps[:], xh[:, c * CHUNK:(c + 1) * CHUNK], m2t[:],
                        start=True, stop=True,
                    )
                    # clamp to [0, 65.5] in one dual-op pass, PSUM -> SBUF
                    nc.vector.tensor_scalar(
                        coor[:, co:co + 3 * G], ps[:], 0.0, 65.5,
                        mybir.AluOpType.max, mybir.AluOpType.min,
                    )

                # floor via the DVE's round-to-nearest f32->i32 cast:
                # c = rne(s - 0.5); frac = s - c compensates the half-integer
                # ties (frac lands on 0.0 or 1.0, lerp stays exact)
                cm = pool.tile([128, SUB_PER_SUPER * 3 * G], FP32, tag="cm")
                cint = pool.tile([128, SUB_PER_SUPER * 3 * G], I32, tag="cint")
                nc.vector.tensor_scalar(cm[:], coor[:], -0.5, None, add)
                nc.vector.tensor_copy(cint[:], cm[:])
                nc.vector.tensor_copy(cflt[:], cint[:])
                nc.vector.tensor_tensor(frac[:], coor[:], cflt[:], subtract)
                # 1 - frac
                nc.vector.tensor_scalar(gfrac[:], frac[:], -1.0, 1.0, mult, add)

                # cell index: (cz*66 + cy)*66 + cx   (exact in f32)
                for c in range(SUB_PER_SUPER):
                    co = c * 3 * G
                    io = c * G
                    cx = cflt[:, co:co + G]
                    cy = cflt[:, co + G:co + 2 * G]
                    cz = cflt[:, co + 2 * G:co + 3 * G]
                    nc.vector.scalar_tensor_tensor(
                        idxf[:, io:io + G], cz, 66.0, cy, mult, add)
                    nc.vector.scalar_tensor_tensor(
                        idxf[:, io:io + G], idxf[:, io:io + G], 66.0, cx, mult, add)
                nc.vector.tensor_copy(idx[:], idxf[:])
                nc.vector.tensor_tensor(
                    idx[:], idx[:],
                    goff[:, None, :].to_broadcast([128, SUB_PER_SUPER, G]),
                    add,
                )

                # ---- gather: one 32B pack per (point, grid) ----
                # HW indirect DMA consumes ONE offset per dest partition, so
                # issue one call per idx column (128 packs each).
                gat = pool.tile([128, SUB_PER_SUPER * G * 16], BF16, tag="gat")
                for k in range(SUB_PER_SUPER * G):
                    nc.gpsimd.indirect_dma_start(
                        out=gat[:, k * 16:(k + 1) * 16],
                        out_offset=None,
                        in_=v8_ext[:],
                        in_offset=bass.IndirectOffsetOnAxis(
                            ap=idx[:, k:k + 1], axis=0),
                    )

                if debug and s == 0:
                    nc.sync.dma_start(out=dbg["coor"][:], in_=coor[:])
                    nc.sync.dma_start(out=dbg["frac"][:], in_=frac[:])
                    nc.sync.dma_start(out=dbg["idx"][:], in_=idx[:])
                    nc.sync.dma_start(out=dbg["gat"][:], in_=gat[:])
                optile = pool.tile([1, SUPER], FP32, tag="optile")
                ops = pp1.tile([1, SUPER], FP32, tag="ps_out")
                for c in range(SUB_PER_SUPER):
                    co = c * 3 * G
                    fx = frac[:, co:co + G]
                    fy = frac[:, co + G:co + 2 * G]
                    fz = frac[:, co + 2 * G:co + 3 * G]
                    gx = gfrac[:, co:co + G]
                    gy = gfrac[:, co + G:co + 2 * G]
                    gz = gfrac[:, co + 2 * G:co + 3 * G]

                    gv = gat[:, c * G * 16:(c + 1) * G * 16].rearrange(
                        "p (g dz r) -> p g dz r", g=G, dz=2, r=8)
                    tz = pool.tile([128, G, 8], FP32, tag="tz")
                    tz2 = pool.tile([128, G, 8], FP32, tag="tz2")
                    bz = lambda t: t[:, :, None].to_broadcast([128, G, 8])
                    nc.vector.tensor_tensor(tz[:], gv[:, :, 0, :], bz(gz), mult)
                    nc.vector.tensor_tensor(tz2[:], gv[:, :, 1, :], bz(fz), mult)
                    nc.vector.tensor_tensor(tz[:], tz[:], tz2[:], add)

                    tv = tz[:].rearrange("p g (dy r) -> p g dy r", dy=2)
                    ty = pool.tile([128, G, 4], FP32, tag="ty")
                    ty2 = pool.tile([128, G, 4], FP32, tag="ty2")
                    by = lambda t: t[:, :, None].to_broadcast([128, G, 4])
                    nc.vector.tensor_tensor(ty[:], tv[:, :, 0, :], by(gy), mult)
                    nc.vector.tensor_tensor(ty2[:], tv[:, :, 1, :], by(fy), mult)
                    nc.vector.tensor_tensor(ty[:], ty[:], ty2[:], add)

                    tw = ty[:].rearrange("p g (dx r) -> p g dx r", dx=2)
                    feats = pool.tile([128, G, 2], BF16, tag="feats")
                    tx1 = pool.tile([128, G, 2], FP32, tag="tx1")
                    tx2 = pool.tile([128, G, 2], FP32, tag="tx2")
                    bx = lambda t: t[:, :, None].to_broadcast([128, G, 2])
                    nc.vector.tensor_tensor(tx1[:], tw[:, :, 0, :], bx(gx), mult)
                    nc.vector.tensor_tensor(tx2[:], tw[:, :, 1, :], bx(fx), mult)
                    nc.vector.tensor_tensor(feats[:], tx1[:], tx2[:], add)
                    if debug and s == 0 and c == 0:
                        nc.sync.dma_start(
                            out=dbg["feats"][:],
                            in_=feats[:].rearrange("p g r -> p (g r)"))

                    # transpose feats [128 pts, 128 gf] -> [128 gf, 128 pts]
                    ftp = pp.tile([128, 128], BF16, tag="ps_ft")
                    nc.tensor.transpose(
                        ftp[:], feats[:].rearrange("p g r -> p (g r)"), ident[:])
                    featsT = pool.tile([128, 128], BF16, tag="featsT")
                    nc.scalar.activation(featsT[:], ftp[:], Copy)
                    if debug and s == 0 and c == 0:
                        nc.sync.dma_start(out=dbg["featsT"][:], in_=featsT[:])

                    # MLP
                    h0p = pp.tile([64, 128], FP32, tag="ps_mlp")
                    nc.tensor.matmul(h0p[:], w0[:], featsT[:], start=True, stop=True)
                    h0 = pool.tile([64, 128], BF16, tag="h0")
                    nc.scalar.activation(h0[:], h0p[:], Relu, bias=b0[:])
                    h1p = pp.tile([64, 128], FP32, tag="ps_mlp")
                    nc.tensor.matmul(h1p[:], w1[:], h0[:], start=True, stop=True)
                    h1 = pool.tile([64, 128], BF16, tag="h1")
                    nc.scalar.activation(h1[:], h1p[:], Relu, bias=b1[:])
                    nc.tensor.matmul(
                        ops[:1, c * CHUNK:(c + 1) * CHUNK], w2[:], h1[:],
                        start=True, stop=True,
                    )

                nc.vector.tensor_scalar(optile[:], ops[:], b2[:], None, add)
                nc.sync.dma_start(
                    out=out_ext[s * SUPER:(s + 1) * SUPER, :], in_=optile[:]
                )

    nc.compile()
    return nc


def host_prep(x, transformation_matrices, feature_grids, W0, b0, W1, b1, W2, b2):
    import ml_dtypes

    M = np.asarray(transformation_matrices, np.float32)  # [G,4,4]
    fg = np.asarray(feature_grids, np.float32)           # [G,F,R,R,R]
    x = np.asarray(x, np.float32)                        # [N,3]

    # transform matrix -> padded grid coords: s_ax = 31.5*p_ax + 32.5
    # p_ax = M[g, ax, :] . [x y z 1]
    M2 = 31.5 * M[:, :3, :]            # [G, 3, 4]
    M2[:, :, 3] += 32.5
    # m2t[k, ax*G + g] = M2[g, ax, k]
    m2t = np.ascontiguousarray(M2.transpose(2, 1, 0).reshape(4, 3 * G))

    # V8 table: padded grid, then per-cell 2x2x2 corner window x F features
    v8 = np.empty((G, CELLS, 16), dtype=ml_dtypes.bfloat16)
    for g in range(G):
        pad = np.zeros((R + 3, R + 3, R + 3, F), np.float32)  # 67^3, orig at +1
        pad[1:R + 1, 1:R + 1, 1:R + 1, :] = fg[g].transpose(1, 2, 3, 0)
        w = np.lib.stride_tricks.sliding_window_view(pad, (2, 2, 2), axis=(0, 1, 2))
        # w: [67-1=66, 66, 66, F, 2, 2, 2] -> [cz, cy, cx, dz, dy, dx, f]
        v8[g] = (
            w.transpose(0, 1, 2, 4, 5, 6, 3)
            .reshape(CELLS, 16)
            .astype(ml_dtypes.bfloat16)
        )
    v8 = v8.reshape(G * CELLS, 16)

    goff = np.broadcast_to(
        (np.arange(G, dtype=np.int64) * CELLS).astype(np.int32), (128, G)
    ).copy()

    xh_full = np.concatenate([x, np.ones((N, 1), np.float32)], axis=1)  # [N,4]

    prep = dict(
        m2t=m2t,
        v8=v8,
        goff=goff,
        w0=np.asarray(W0, np.float32).astype(ml_dtypes.bfloat16),
        w1=np.asarray(W1, np.float32).astype(ml_dtypes.bfloat16),
        w2=np.asarray(W2, np.float32).astype(ml_dtypes.bfloat16),
        b0=np.asarray(b0, np.float32).reshape(64, 1),
        b1=np.asarray(b1, np.float32).reshape(64, 1),
        b2=np.asarray(b2, np.float32).reshape(1, 1),
    )
    return prep, xh_full


def run(inputs, trace=False):
    from concourse.bass_utils import run_bass_kernel_spmd

    prep, xh_full = host_prep(**inputs)
    nc = build_bass(NP)
    in_maps = []
    for c in range(NCORES):
        m = dict(prep)
        m["xh"] = np.ascontiguousarray(xh_full[c * NP:(c + 1) * NP, :].T)
        in_maps.append(m)
    res = run_bass_kernel_spmd(nc, in_maps, core_ids=list(range(NCORES)), trace=trace)
    out = np.concatenate([res.results[c]["out"] for c in range(NCORES)], axis=0)
    return out.astype(np.float32), res.exec_time_ns


def kernel(x, transformation_matrices, feature_grids, W0, b0, W1, b1, W2, b2):
    out, _ = run(
        dict(x=x, transformation_matrices=transformation_matrices,
             feature_grids=feature_grids, W0=W0, b0=b0, W1=W1, b1=b1, W2=W2, b2=b2)
    )
    return out


# revision 16
# speedup vs baseline: 1.1847x; 1.1847x over previous
"""AMGSRN (multi-grid encoding + MLP decoder) on 8 Trainium2 NeuronCores.

Strategy (data-parallel over the point batch, grids/weights replicated):
  - Host precomputes a "V8" corner table per grid: for every padded cell c
    (66^3 cells per grid) it stores the 16 values {corner(dz,dy,dx), feature f}
    contiguously (32B in bf16).  One trilinear sample then needs exactly ONE
    32B indirect-DMA gather per (point, grid).
  - Device per chunk of 128 points: PE computes the affine transform straight
    into padded-grid coordinates, ACT clamps, DVE computes cell indices +
    fractional weights, one indirect DMA gathers 128x64 corner packs, DVE does
    the factored trilinear combine (z, then y, then x), PE transposes the
    [point, feature] tile and runs the 128->64->64->1 MLP with fused
    bias+ReLU on ACT.
  - No collectives: each core owns 32768 points end-to-end.
"""

import sys

sys.path.insert(0, "/opt/trn_rl_repo")

import numpy as np

import concourse.bass as bass
import concourse.bacc as bacc_mod
import concourse.mybir as mybir
import concourse.tile as tile
from concourse.masks import make_identity

# problem constants (hardcoded per harness rules)
G = 64          # grids
F = 2           # features per grid
R = 64          # grid resolution
N = 262144      # total points
NCORES = 8
NP = N // NCORES            # points per core
PAD = 66                    # padded cell axis (cell' in [0,65])
CELLS = PAD * PAD * PAD     # 287496 cells per grid
CHUNK = 128
SUB_PER_SUPER = 4
SUPER = CHUNK * SUB_PER_SUPER   # 512 points
NSUPER = NP // SUPER

FP32 = mybir.dt.float32
BF16 = mybir.dt.bfloat16
I32 = mybir.dt.int32


def _register_const(nc, dtype, value):
    t = nc.alloc_sbuf_tensor(f"const-{dtype.name}-{value}", [128, 1], dtype)
    nc.gpsimd.memset(t.ap(), value)
    nc.const_aps.aps[(dtype, value)] = t.ap()


def build_bass(np_points=NP, debug=False):
    nsuper = np_points // SUPER
    nc = bacc_mod.Bacc(num_swdge_queues=4)
    dbg = {}
    if debug:
        dbg["coor"] = nc.declare_dram_parameter("dbg_coor", [128, SUB_PER_SUPER * 3 * G], FP32, isOutput=True)
        dbg["frac"] = nc.declare_dram_parameter("dbg_frac", [128, SUB_PER_SUPER * 3 * G], FP32, isOutput=True)
        dbg["idx"] = nc.declare_dram_parameter("dbg_idx", [128, SUB_PER_SUPER * G], I32, isOutput=True)
        dbg["gat"] = nc.declare_dram_parameter("dbg_gat", [128, SUB_PER_SUPER * G * 16], BF16, isOutput=True)
        dbg["feats"] = nc.declare_dram_parameter("dbg_feats", [128, 128], BF16, isOutput=True)
        dbg["featsT"] = nc.declare_dram_parameter("dbg_featsT", [128, 128], BF16, isOutput=True)

    xh_ext = nc.declare_dram_parameter("xh", [4, np_points], FP32, isOutput=False)
    m2t_ext = nc.declare_dram_parameter("m2t", [4, 3 * G], FP32, isOutput=False)
    v8_ext = nc.declare_dram_parameter("v8", [G * CELLS, 16], BF16, isOutput=False)
    goff_ext = nc.declare_dram_parameter("goff", [128, G], I32, isOutput=False)
    w0_ext = nc.declare_dram_parameter("w0", [G * F, 64], BF16, isOutput=False)
    w1_ext = nc.declare_dram_parameter("w1", [64, 64], BF16, isOutput=False)
    w2_ext = nc.declare_dram_parameter("w2", [64, 1], BF16, isOutput=False)
    b0_ext = nc.declare_dram_parameter("b0", [64, 1], FP32, isOutput=False)
    b1_ext = nc.declare_dram_parameter("b1", [64, 1], FP32, isOutput=False)
    b2_ext = nc.declare_dram_parameter("b2", [1, 1], FP32, isOutput=False)
    out_ext = nc.declare_dram_parameter("out", [np_points, 1], FP32, isOutput=True)

    Relu = mybir.ActivationFunctionType.Relu
    Copy = mybir.ActivationFunctionType.Copy
    mult = mybir.AluOpType.mult
    add = mybir.AluOpType.add
    subtract = mybir.AluOpType.subtract

    with tile.TileContext(nc) as tc:
        with (
            tc.tile_pool(name="const", bufs=1) as cpool,
            tc.tile_pool(name="sb", bufs=2) as pool,
            tc.tile_pool(name="psum", bufs=2, space="PSUM") as pp,
            tc.tile_pool(name="psum1", bufs=1, space="PSUM") as pp1,
        ):
            # ---- persistent constants ----
            m2t = cpool.tile([4, 3 * G], FP32)
            nc.sync.dma_start(out=m2t[:], in_=m2t_ext[:])
            goff = cpool.tile([128, G], I32)
            nc.sync.dma_start(out=goff[:], in_=goff_ext[:])
            w0 = cpool.tile([G * F, 64], BF16)
            nc.sync.dma_start(out=w0[:], in_=w0_ext[:])
            w1 = cpool.tile([64, 64], BF16)
            nc.sync.dma_start(out=w1[:], in_=w1_ext[:])
            w2 = cpool.tile([64, 1], BF16)
            nc.sync.dma_start(out=w2[:], in_=w2_ext[:])
            b0 = cpool.tile([64, 1], FP32)
            nc.sync.dma_start(out=b0[:], in_=b0_ext[:])
            b1 = cpool.tile([64, 1], FP32)
            nc.sync.dma_start(out=b1[:], in_=b1_ext[:])
            b2 = cpool.tile([1, 1], FP32)
            nc.sync.dma_start(out=b2[:], in_=b2_ext[:])
            ident = cpool.tile([128, 128], BF16)
            make_identity(nc, ident[:])

            for s in range(nsuper):
                # ---- load this super-chunk's points ----
                xh = pool.tile([4, SUPER], FP32, tag="xh")
                nc.sync.dma_start(out=xh[:], in_=xh_ext[:, s * SUPER:(s + 1) * SUPER])

                # per-super working tiles
                coor = pool.tile([128, SUB_PER_SUPER * 3 * G], FP32, tag="coor")
                cflt = pool.tile([128, SUB_PER_SUPER * 3 * G], FP32, tag="cflt")
                frac = pool.tile([128, SUB_PER_SUPER * 3 * G], FP32, tag="frac")
                gfrac = pool.tile([128, SUB_PER_SUPER * 3 * G], FP32, tag="gfrac")
                idx = pool.tile([128, SUB_PER_SUPER * G], I32, tag="idx")
                idxf = pool.tile([128, SUB_PER_SUPER * G], FP32, tag="idxf")

                for c in range(SUB_PER_SUPER):
                    co = c * 3 * G  # column offset in the (sub, ax, g) layout
                    # transform: [4,128].T @ [4,192] -> [128, 192] (ax-major: x|y|z)
                    ps = pp.tile([128, 3 * G], FP32, tag="ps_tr")
                    nc.tensor.matmul(
                        ps[:], xh[:, c * CHUNK:(c + 1) * CHUNK], m2t[:],
                        start=True, stop=True,
                    )
                    # clamp to [0, 65.5] in one dual-op pass, PSUM -> SBUF
                    nc.vector.tensor_scalar(
                        coor[:, co:co + 3 * G], ps[:], 0.0, 65.5,
                        mybir.AluOpType.max, mybir.AluOpType.min,
                    )

                # floor via the DVE's round-to-nearest f32->i32 cast:
                # c = rne(s - 0.5); frac = s - c compensates the half-integer
                # ties (frac lands on 0.0 or 1.0, lerp stays exact)
                cm = pool.tile([128, SUB_PER_SUPER * 3 * G], FP32, tag="cm")
                cint = pool.tile([128, SUB_PER_SUPER * 3 * G], I32, tag="cint")
                nc.vector.tensor_scalar(cm[:], coor[:], -0.5, None, add)
                nc.vector.tensor_copy(cint[:], cm[:])
                nc.vector.tensor_copy(cflt[:], cint[:])
                nc.vector.tensor_tensor(frac[:], coor[:], cflt[:], subtract)
                # 1 - frac
                nc.vector.tensor_scalar(gfrac[:], frac[:], -1.0, 1.0, mult, add)

                # cell index: (cz*66 + cy)*66 + cx   (exact in f32)
                for c in range(SUB_PER_SUPER):
                    co = c * 3 * G
                    io = c * G
                    cx = cflt[:, co:co + G]
                    cy = cflt[:, co + G:co + 2 * G]
                    cz = cflt[:, co + 2 * G:co + 3 * G]
                    nc.vector.scalar_tensor_tensor(
                        idxf[:, io:io + G], cz, 66.0, cy, mult, add)
                    nc.vector.scalar_tensor_tensor(
                        idxf[:, io:io + G], idxf[:, io:io + G], 66.0, cx, mult, add)
                nc.vector.tensor_copy(idx[:], idxf[:])
                nc.vector.tensor_tensor(
                    idx[:], idx[:],
                    goff[:, None, :].to_broadcast([128, SUB_PER_SUPER, G]),
                    add,
                )

                # ---- gather: one 32B pack per (point, grid) ----
                # HW indirect DMA consumes ONE offset per dest partition, so
                # issue one call per idx column (128 packs each).
                gat = pool.tile([128, SUB_PER_SUPER * G * 16], BF16, tag="gat")
                for k in range(SUB_PER_SUPER * G):
                    gi = nc.gpsimd.indirect_dma_start(
                        out=gat[:, k * 16:(k + 1) * 16],
                        out_offset=None,
                        in_=v8_ext[:],
                        in_offset=bass.IndirectOffsetOnAxis(
                            ap=idx[:, k:k + 1], axis=0),
                    )
                    q = k % 4
                    if q:
                        gi.ins.queue = f"qPoolDynamic{q}"

                if debug and s == 0:
                    nc.sync.dma_start(out=dbg["coor"][:], in_=coor[:])
                    nc.sync.dma_start(out=dbg["frac"][:], in_=frac[:])
                    nc.sync.dma_start(out=dbg["idx"][:], in_=idx[:])
                    nc.sync.dma_start(out=dbg["gat"][:], in_=gat[:])
                optile = pool.tile([1, SUPER], FP32, tag="optile")
                ops = pp1.tile([1, SUPER], FP32, tag="ps_out")
                for c in range(SUB_PER_SUPER):
                    co = c * 3 * G
                    fx = frac[:, co:co + G]
                    fy = frac[:, co + G:co + 2 * G]
                    fz = frac[:, co + 2 * G:co + 3 * G]
                    gx = gfrac[:, co:co + G]
                    gy = gfrac[:, co + G:co + 2 * G]
                    gz = gfrac[:, co + 2 * G:co + 3 * G]

                    gv = gat[:, c * G * 16:(c + 1) * G * 16].rearrange(
                        "p (g dz r) -> p g dz r", g=G, dz=2, r=8)
                    tz = pool.tile([128, G, 8], FP32, tag="tz")
                    tz2 = pool.tile([128, G, 8], FP32, tag="tz2")
                    bz = lambda t: t[:, :, None].to_broadcast([128, G, 8])
                    nc.vector.tensor_tensor(tz[:], gv[:, :, 0, :], bz(gz), mult)
                    nc.vector.tensor_tensor(tz2[:], gv[:, :, 1, :], bz(fz), mult)
                    nc.vector.tensor_tensor(tz[:], tz[:], tz2[:], add)

                    tv = tz[:].rearrange("p g (dy r) -> p g dy r", dy=2)
                    ty = pool.tile([128, G, 4], FP32, tag="ty")
                    ty2 = pool.tile([128, G, 4], FP32, tag="ty2")
                    by = lambda t: t[:, :, None].to_broadcast([128, G, 4])
                    nc.vector.tensor_tensor(ty[:], tv[:, :, 0, :], by(gy), mult)
                    nc.vector.tensor_tensor(ty2[:], tv[:, :, 1, :], by(fy), mult)
                    nc.vector.tensor_tensor(ty[:], ty[:], ty2[:], add)

                    tw = ty[:].rearrange("p g (dx r) -> p g dx r", dx=2)
                    feats = pool.tile([128, G, 2], BF16, tag="feats")
                    tx1 = pool.tile([128, G, 2], FP32, tag="tx1")
                    tx2 = pool.tile([128, G, 2], FP32, tag="tx2")
                    bx = lambda t: t[:, :, None].to_broadcast([128, G, 2])
                    nc.vector.tensor_tensor(tx1[:], tw[:, :, 0, :], bx(gx), mult)
                    nc.vector.tensor_tensor(tx2[:], tw[:, :, 1, :], bx(fx), mult)
                    nc.vector.tensor_tensor(feats[:], tx1[:], tx2[:], add)
                    if debug and s == 0 and c == 0:
                        nc.sync.dma_start(
                            out=dbg["feats"][:],
                            in_=feats[:].rearrange("p g r -> p (g r)"))

                    # transpose feats [128 pts, 128 gf] -> [128 gf, 128 pts]
                    ftp = pp.tile([128, 128], BF16, tag="ps_ft")
                    nc.tensor.transpose(
                        ftp[:], feats[:].rearrange("p g r -> p (g r)"), ident[:])
                    featsT = pool.tile([128, 128], BF16, tag="featsT")
                    nc.scalar.activation(featsT[:], ftp[:], Copy)
                    if debug and s == 0 and c == 0:
                        nc.sync.dma_start(out=dbg["featsT"][:], in_=featsT[:])

                    # MLP
                    h0p = pp.tile([64, 128], FP32, tag="ps_mlp")
                    nc.tensor.matmul(h0p[:], w0[:], featsT[:], start=True, stop=True)
                    h0 = pool.tile([64, 128], BF16, tag="h0")
                    nc.scalar.activation(h0[:], h0p[:], Relu, bias=b0[:])
                    h1p = pp.tile([64, 128], FP32, tag="ps_mlp")
                    nc.tensor.matmul(h1p[:], w1[:], h0[:], start=True, stop=True)
                    h1 = pool.tile([64, 128], BF16, tag="h1")
                    nc.scalar.activation(h1[:], h1p[:], Relu, bias=b1[:])
                    nc.tensor.matmul(
                        ops[:1, c * CHUNK:(c + 1) * CHUNK], w2[:], h1[:],
                        start=True, stop=True,
                    )

                nc.vector.tensor_scalar(optile[:], ops[:], b2[:], None, add)
                nc.sync.dma_start(
                    out=out_ext[s * SUPER:(s + 1) * SUPER, :], in_=optile[:]
                )

    nc.compile()
    return nc


def host_prep(x, transformation_matrices, feature_grids, W0, b0, W1, b1, W2, b2):
    import ml_dtypes

    M = np.asarray(transformation_matrices, np.float32)  # [G,4,4]
    fg = np.asarray(feature_grids, np.float32)           # [G,F,R,R,R]
    x = np.asarray(x, np.float32)                        # [N,3]

    # transform matrix -> padded grid coords: s_ax = 31.5*p_ax + 32.5
    # p_ax = M[g, ax, :] . [x y z 1]
    M2 = 31.5 * M[:, :3, :]            # [G, 3, 4]
    M2[:, :, 3] += 32.5
    # m2t[k, ax*G + g] = M2[g, ax, k]
    m2t = np.ascontiguousarray(M2.transpose(2, 1, 0).reshape(4, 3 * G))

    # V8 table: padded grid, then per-cell 2x2x2 corner window x F features
    v8 = np.empty((G, CELLS, 16), dtype=ml_dtypes.bfloat16)
    for g in range(G):
        pad = np.zeros((R + 3, R + 3, R + 3, F), np.float32)  # 67^3, orig at +1
        pad[1:R + 1, 1:R + 1, 1:R + 1, :] = fg[g].transpose(1, 2, 3, 0)
        w = np.lib.stride_tricks.sliding_window_view(pad, (2, 2, 2), axis=(0, 1, 2))
        # w: [67-1=66, 66, 66, F, 2, 2, 2] -> [cz, cy, cx, dz, dy, dx, f]
        v8[g] = (
            w.transpose(0, 1, 2, 4, 5, 6, 3)
            .reshape(CELLS, 16)
            .astype(ml_dtypes.bfloat16)
        )
    v8 = v8.reshape(G * CELLS, 16)

    goff = np.broadcast_to(
        (np.arange(G, dtype=np.int64) * CELLS).astype(np.int32), (128, G)
    ).copy()

    xh_full = np.concatenate([x, np.ones((N, 1), np.float32)], axis=1)  # [N,4]

    prep = dict(
        m2t=m2t,
        v8=v8,
        goff=goff,
        w0=np.asarray(W0, np.float32).astype(ml_dtypes.bfloat16),
        w1=np.asarray(W1, np.float32).astype(ml_dtypes.bfloat16),
        w2=np.asarray(W2, np.float32).astype(ml_dtypes.bfloat16),
        b0=np.asarray(b0, np.float32).reshape(64, 1),
        b1=np.asarray(b1, np.float32).reshape(64, 1),
        b2=np.asarray(b2, np.float32).reshape(1, 1),
    )
    return prep, xh_full


def run(inputs, trace=False):
    from concourse.bass_utils import run_bass_kernel_spmd

    prep, xh_full = host_prep(**inputs)
    nc = build_bass(NP)
    in_maps = []
    for c in range(NCORES):
        m = dict(prep)
        m["xh"] = np.ascontiguousarray(xh_full[c * NP:(c + 1) * NP, :].T)
        in_maps.append(m)
    res = run_bass_kernel_spmd(nc, in_maps, core_ids=list(range(NCORES)), trace=trace)
    out = np.concatenate([res.results[c]["out"] for c in range(NCORES)], axis=0)
    return out.astype(np.float32), res.exec_time_ns


def kernel(x, transformation_matrices, feature_grids, W0, b0, W1, b1, W2, b2):
    out, _ = run(
        dict(x=x, transformation_matrices=transformation_matrices,
             feature_grids=feature_grids, W0=W0, b0=b0, W1=W1, b1=b1, W2=W2, b2=b2)
    )
    return out
